# revision 1
# baseline (speedup 1.0000x reference)
"""MultiHeadAttention TRN2 kernel: B=8, S=1024, D=1024, H=16, D_H=64.

Sharding: batch-parallel, one batch element per NeuronCore (8 cores), no
collectives. Per core:

  - Q/K projections and QK^T run as bf16 "split-3" (hi/lo decomposition:
    x*w ~= xh*wh + xh*wl + xl*wh, fp32 PSUM accumulation) for near-fp32
    accuracy -- the scores here reach +-18000 and softmax is ~one-hot, so
    plain bf16 corrupts rows whose top-2 score gap is small.
  - Softmax: DVE rowmax (negated) -> ACT exp(scale*s + bias) with fused
    rowsum (accum_out); normalization folded into the transpose:
    pT = e_blk.T @ diag(1/rowsum) as a plain PE matmul.
  - PV and the output projection run in bf16 with col/row packing.
  - Bias add via rank-1 (K=1) matmul of ones x b_out.

Everything is hardcoded for the shapes above.
"""
import numpy as np
import ml_dtypes

BF = ml_dtypes.bfloat16
S = 1024
D = 1024
H = 16
DH = 64
NKT = 8          # k tiles of 128 over D
NST = 8          # s tiles of 128 over S
NPAIR = 8        # head pairs
SCALE = 0.125

_CACHE = {}


def _build_nc():
    import concourse.bass as bass
    import concourse.mybir as mybir
    import concourse.tile as tile
    from concourse import bacc
    from concourse.masks import make_identity

    f32 = mybir.dt.float32
    bf16 = mybir.dt.bfloat16

    nc = bacc.Bacc("TRN2", target_bir_lowering=False, debug=False)

    def din(name):
        return nc.dram_tensor(name, [D, S], bf16, kind="ExternalInput")

    xq_hi, xq_lo = din("xq_hi"), din("xq_lo")
    xk_hi, xk_lo = din("xk_hi"), din("xk_lo")
    xv = din("xv")
    wq_hi, wq_lo = din("wq_hi"), din("wq_lo")
    wk_hi, wk_lo = din("wk_hi"), din("wk_lo")
    wv = din("wv")
    woutT = din("woutT")
    bvec = nc.dram_tensor("bvec", [1, D], bf16, kind="ExternalInput")
    out = nc.dram_tensor("out", [S, D], f32, kind="ExternalOutput")

    with tile.TileContext(nc) as tc:
        with tc.tile_pool(name="persist", bufs=1) as pp:
            # projection outputs, persistent across phase B
            q_hi = [pp.tile([128, S], bf16, name=f"q_hi{g}") for g in range(NPAIR)]
            q_lo = [pp.tile([128, S], bf16, name=f"q_lo{g}") for g in range(NPAIR)]
            k_hi = [pp.tile([128, S], bf16, name=f"k_hi{g}") for g in range(NPAIR)]
            k_lo = [pp.tile([128, S], bf16, name=f"k_lo{g}") for g in range(NPAIR)]
            v_sb = [pp.tile([128, D], bf16, name=f"v_sb{g}") for g in range(NST)]
            oT_sb = [pp.tile([128, S], bf16, name=f"oT_sb{j}") for j in range(NPAIR)]
            ident = pp.tile([128, 128], bf16, name="ident")
            make_identity(nc, ident[:])

            # ---------------- Phase A: projections ----------------
            # q/k: qT[(2h dims), s] = sum_k W[k, dims] * XT[k, s]  (split-3)
            for pname, whi_d, wlo_d, xhi_d, xlo_d, ohi, olo in (
                ("q", wq_hi, wq_lo, xq_hi, xq_lo, q_hi, q_lo),
                ("k", wk_hi, wk_lo, xk_hi, xk_lo, k_hi, k_lo),
            ):
                with (
                    tc.tile_pool(name=f"pa_{pname}", bufs=1) as pa,
                    tc.tile_pool(name=f"pa_ps_{pname}", bufs=3, space="PSUM") as pap,
                ):
                    wh = [pa.tile([128, D], bf16, name=f"{pname}wh{t}") for t in range(NKT)]
                    wl = [pa.tile([128, D], bf16, name=f"{pname}wl{t}") for t in range(NKT)]
                    xh = [pa.tile([128, S], bf16, name=f"{pname}xh{t}") for t in range(NKT)]
                    xl = [pa.tile([128, S], bf16, name=f"{pname}xl{t}") for t in range(NKT)]
                    for t in range(NKT):
                        sl = slice(t * 128, (t + 1) * 128)
                        nc.sync.dma_start(wh[t][:], whi_d[sl, :])
                        nc.sync.dma_start(wl[t][:], wlo_d[sl, :])
                        nc.sync.dma_start(xh[t][:], xhi_d[sl, :])
                        nc.sync.dma_start(xl[t][:], xlo_d[sl, :])
                    for g in range(NPAIR):
                        msl = slice(g * 128, (g + 1) * 128)
                        for ch in range(2):
                            nsl = slice(ch * 512, (ch + 1) * 512)
                            ps = pap.tile([128, 512], f32, name=f"ps_{pname}{g}_{ch}",
                                          tag="pa_ps")
                            n = 0
                            for t in range(NKT):
                                for lw, rx in ((wh[t], xh[t]), (wh[t], xl[t]),
                                               (wl[t], xh[t])):
                                    nc.tensor.matmul(ps[:], lw[:, msl], rx[:, nsl],
                                                     start=(n == 0),
                                                     stop=(n == 3 * NKT - 1))
                                    n += 1
                            nc.vector.tensor_copy(ohi[g][:, nsl], ps[:])
                            nc.vector.tensor_sub(olo[g][:, nsl], ps[:], ohi[g][:, nsl])

            # v: v[s, j] = sum_k XvT[k, s] * Wv[k, j]   (bf16 single)
            with (
                tc.tile_pool(name="pa_v", bufs=1) as pa,
                tc.tile_pool(name="pa_ps_v", bufs=3, space="PSUM") as pap,
            ):
                wvt = [pa.tile([128, D], bf16, name=f"vw{t}") for t in range(NKT)]
                xvt = [pa.tile([128, S], bf16, name=f"vx{t}") for t in range(NKT)]
                for t in range(NKT):
                    sl = slice(t * 128, (t + 1) * 128)
                    nc.sync.dma_start(wvt[t][:], wv[sl, :])
                    nc.sync.dma_start(xvt[t][:], xv[sl, :])
                for g in range(NST):
                    msl = slice(g * 128, (g + 1) * 128)
                    for ch in range(2):
                        nsl = slice(ch * 512, (ch + 1) * 512)
                        ps = pap.tile([128, 512], f32, name=f"ps_v{g}_{ch}",
                                      tag="pa_ps")
                        for t in range(NKT):
                            nc.tensor.matmul(ps[:], xvt[t][:, msl], wvt[t][:, nsl],
                                             start=(t == 0), stop=(t == NKT - 1))
                        nc.vector.tensor_copy(v_sb[g][:, nsl], ps[:])

            # ---------------- Phase B: attention per head-pair ----------------
            with (
                tc.tile_pool(name="pb", bufs=1) as pb,
                tc.tile_pool(name="pb_ps", bufs=1, space="PSUM") as pbp,
            ):
                for pair in range(NPAIR):
                    pT = {}
                    for hp in range(2):  # head 2*pair + hp
                        poff = 64 * hp
                        psl = slice(poff, poff + 64)
                        e_t = []
                        dg_t = []
                        for qt in range(NST):
                            qsl = slice(qt * 128, (qt + 1) * 128)
                            ps_s = pbp.tile([128, S], f32,
                                            name=f"ps_s{pair}_{hp}_{qt}",
                                            tag="ps_s", bufs=2)
                            for ch in range(2):
                                nsl = slice(ch * 512, (ch + 1) * 512)
                                n = 0
                                for lq, rk in ((q_hi, k_hi), (q_hi, k_lo),
                                               (q_lo, k_hi)):
                                    nc.tensor.matmul(
                                        ps_s[:, nsl],
                                        lq[pair][psl, qsl], rk[pair][psl, nsl],
                                        start=(n == 0), stop=(n == 2),
                                        tile_position=(poff, 0))
                                    n += 1
                            negm = pb.tile([128, 1], f32,
                                           name=f"negm{pair}_{hp}_{qt}",
                                           tag="negm", bufs=2)
                            biasv = pb.tile([128, 1], f32,
                                            name=f"biasv{pair}_{hp}_{qt}",
                                            tag="biasv", bufs=2)
                            r_t = pb.tile([128, 1], f32,
                                          name=f"r{pair}_{hp}_{qt}",
                                          tag="r", bufs=2)
                            rinv = pb.tile([128, 1], f32,
                                           name=f"rinv{pair}_{hp}_{qt}",
                                           tag="rinv", bufs=2)
                            e_sb = pb.tile([128, S], bf16,
                                           name=f"e{pair}_{hp}_{qt}",
                                           tag=f"e{qt}", bufs=2)
                            diag = pb.tile([128, 128], bf16,
                                           name=f"diag{pair}_{hp}_{qt}",
                                           tag=f"diag{qt}", bufs=2)
                            nc.vector.tensor_reduce(
                                negm[:], ps_s[:], axis=mybir.AxisListType.X,
                                op=mybir.AluOpType.max, negate=True)
                            nc.vector.tensor_scalar_mul(biasv[:], negm[:], SCALE)
                            nc.scalar.activation(
                                e_sb[:], ps_s[:], mybir.ActivationFunctionType.Exp,
                                bias=biasv[:], scale=SCALE, accum_out=r_t[:])
                            nc.vector.reciprocal(rinv[:], r_t[:])
                            nc.vector.tensor_scalar_mul(diag[:], ident[:],
                                                        rinv[:, 0:1])
                            e_t.append(e_sb)
                            dg_t.append(diag)
                        for kb in range(NKT):
                            ksl = slice(kb * 128, (kb + 1) * 128)
                            ps_pT = pbp.tile([128, S], f32,
                                             name=f"ps_pT{pair}_{hp}_{kb}",
                                             tag="ps_pT", bufs=1)
                            for qt in range(NST):
                                qsl = slice(qt * 128, (qt + 1) * 128)
                                nc.tensor.matmul(ps_pT[:, qsl],
                                                 e_t[qt][:, ksl], dg_t[qt][:],
                                                 start=True, stop=True)
                            pt_sb = pb.tile([128, S], bf16,
                                            name=f"pT{pair}_{hp}_{kb}",
                                            tag=f"pT{hp}_{kb}", bufs=1)
                            nc.scalar.copy(pt_sb[:], ps_pT[:])
                            pT[(hp, kb)] = pt_sb
                    # PV for the pair: o^T[dims, s] ; head hp -> psum rows 64*hp
                    for ch in range(2):
                        nsl = slice(ch * 512, (ch + 1) * 512)
                        ps_o = pbp.tile([128, 512], f32,
                                        name=f"ps_o{pair}_{ch}",
                                        tag="ps_o", bufs=2)
                        for hp in range(2):
                            vsl = slice(pair * 128 + 64 * hp,
                                        pair * 128 + 64 * hp + 64)
                            for kb in range(NKT):
                                nc.tensor.matmul(
                                    ps_o[64 * hp:64 * hp + 64, :],
                                    v_sb[kb][:, vsl], pT[(hp, kb)][:, nsl],
                                    start=(kb == 0), stop=(kb == NKT - 1),
                                    tile_position=(0, 64 * hp))
                        nc.scalar.copy(oT_sb[pair][:, nsl], ps_o[:])

            # ---------------- Phase C: output projection ----------------
            with (
                tc.tile_pool(name="pc", bufs=1) as pc,
                tc.tile_pool(name="pc_ps", bufs=2, space="PSUM") as pcp,
            ):
                wo = [pc.tile([128, D], bf16, name=f"wo{t}") for t in range(NKT)]
                for t in range(NKT):
                    nc.sync.dma_start(wo[t][:], woutT[t * 128:(t + 1) * 128, :])
                ones1 = pc.tile([1, 128], bf16, name="ones1")
                nc.gpsimd.memset(ones1[:], 1.0)
                b_sb = pc.tile([1, D], bf16, name="b_sb")
                nc.sync.dma_start(b_sb[:], bvec[:])
                for st in range(NST):
                    ssl = slice(st * 128, (st + 1) * 128)
                    ps_f = pcp.tile([128, D], f32, name=f"ps_f{st}", tag="ps_f",
                                    bufs=2)
                    for ch in range(2):
                        nsl = slice(ch * 512, (ch + 1) * 512)
                        for jt in range(NKT):
                            nc.tensor.matmul(ps_f[:, nsl],
                                             oT_sb[jt][:, ssl], wo[jt][:, nsl],
                                             start=(jt == 0), stop=False)
                        nc.tensor.matmul(ps_f[:, nsl], ones1[:], b_sb[:, nsl],
                                         start=False, stop=True)
                    o_f = pc.tile([128, D], f32, name=f"of{st}", tag="of", bufs=2)
                    nc.vector.tensor_copy(o_f[:], ps_f[:])
                    nc.sync.dma_start(out[ssl, :], o_f[:])

    nc.compile()
    return nc


def _get_nc():
    if "nc" not in _CACHE:
        _CACHE["nc"] = _build_nc()
    return _CACHE["nc"]


def _split(x):
    hi = x.astype(BF)
    lo = (x - hi.astype(np.float32)).astype(BF)
    return np.ascontiguousarray(hi), np.ascontiguousarray(lo)


def kernel(Qs, Ks, Vs, mask_3d, W_Q, W_K, W_V, W_out, b_out):
    if np.asarray(mask_3d).any():
        return _masked_fallback(Qs, Ks, Vs, mask_3d, W_Q, W_K, W_V, W_out, b_out)

    from concourse.bass_utils import run_bass_kernel_spmd

    B = Qs.shape[0]
    Wq = np.ascontiguousarray(W_Q.transpose(1, 0, 2).reshape(D, D))
    Wk = np.ascontiguousarray(W_K.transpose(1, 0, 2).reshape(D, D))
    Wv = np.ascontiguousarray(W_V.transpose(1, 0, 2).reshape(D, D))
    wq_hi, wq_lo = _split(Wq)
    wk_hi, wk_lo = _split(Wk)
    wv_b = Wv.astype(BF)
    woutT = np.ascontiguousarray(W_out.T).astype(BF)
    bvec = b_out.reshape(1, D).astype(BF)

    in_maps = []
    for b in range(B):
        xq = np.ascontiguousarray(Qs[b].T)
        xk = np.ascontiguousarray(Ks[b].T)
        xv = np.ascontiguousarray(Vs[b].T)
        xq_hi, xq_lo = _split(xq)
        xk_hi, xk_lo = _split(xk)
        in_maps.append({
            "xq_hi": xq_hi, "xq_lo": xq_lo,
            "xk_hi": xk_hi, "xk_lo": xk_lo,
            "xv": xv.astype(BF),
            "wq_hi": wq_hi, "wq_lo": wq_lo,
            "wk_hi": wk_hi, "wk_lo": wk_lo,
            "wv": wv_b, "woutT": woutT, "bvec": bvec,
        })

    nc = _get_nc()
    res = run_bass_kernel_spmd(nc, in_maps, core_ids=list(range(B)))
    return np.stack([res.results[b]["out"] for b in range(B)], axis=0)


def _masked_fallback(Qs, Ks, Vs, mask_3d, W_Q, W_K, W_V, W_out, b_out):
    # mask is all-False for this problem's input spec; numpy fallback keeps
    # kernel() total for any other mask.
    B = Qs.shape[0]
    out = np.zeros((B, S, D), np.float32)
    Wq = W_Q.transpose(1, 0, 2).reshape(D, D)
    Wk = W_K.transpose(1, 0, 2).reshape(D, D)
    Wv = W_V.transpose(1, 0, 2).reshape(D, D)
    for b in range(B):
        q = Qs[b] @ Wq
        k = Ks[b] @ Wk
        v = Vs[b] @ Wv
        af = np.zeros((S, D), np.float32)
        for h in range(H):
            sl = slice(h * DH, (h + 1) * DH)
            s = (q[:, sl] @ k[:, sl].T) * SCALE
            s = np.where(mask_3d[b], -1e9, s)
            e = np.exp(s - s.max(-1, keepdims=True))
            af[:, sl] = (e / e.sum(-1, keepdims=True)) @ v[:, sl]
        out[b] = af @ W_out.T + b_out
    return out
